# revision 26
# baseline (speedup 1.0000x reference)
"""Trainium2 Bass kernel for nn_AdaptiveLNN — 2-stage cross-core pipeline.

The recurrent scan is weight-load bound: each layer-step issues 128
LDWEIGHTS+MATMUL pairs (8 H x H matrix passes x 16 chunks of 128x128)
whose cost is independent of the batch-column count N.  The baseline
ran both layers' scans on every core (1024 layer-steps at N=8).  Here:

  - Cores are paired {2p, 2p+1}; pair p owns examples [16p, 16p+16).
  - Even cores scan LAYER 0 (N=16), odd cores scan LAYER 1 + attention.
  - Per 64-step chunk, the even core LN-projects its h0 chunk into
    (I1, X1) and ships it to the odd core via a paired AllGather
    (groups [[0,1],[2,3],[4,5],[6,7]]); the odd core consumes it one
    iteration later (1-chunk pipeline skew).
  - SPMD: every core runs the SAME program; roles differ only in the
    DATA (weights, masks).  The scan input is blended as
    ich = P1_local + rolemask * RX  (rolemask: even=0, odd=1), and odd
    cores reset state after the (zero-input) warmup chunk via a
    multiplicative keep mask.
  - Each core then runs 512+64 layer-steps instead of 1024 -> ~1.8x.

Attention: only q[T-1] needed -> O(T) attention on the odd cores (K
from the per-chunk projections, V recomputed per-example from h1).
"""

import numpy as np

B, T, IN, H, OUT, NH = 64, 512, 256, 512, 256, 4
HD = H // NH
DT = 0.1
UNFOLDS = 6
MIN_TAU, MAX_TAU = 0.1, 10.0
NCORES = 8
NPAIR = 4
BC = B // NPAIR           # 16 examples per pair
HC = H // 128             # 4
P = 128
EPS = 1e-5
CH = 64                   # pipeline chunk (steps)
NCH = T // CH             # 8
TP = T + CH               # padded timeline (576) incl. warmup/drain chunk
GROUPS = [[0, 1], [2, 3], [4, 5], [6, 7]]

_CACHE = {}


# ---------------------------------------------------------------- host packing

def _wT(Wt):
    """(out_f, in_f) -> lhsT sbuf layout (128, nk*out_f):
    [p, kc*out_f + m] = W[m, kc*128 + p]."""
    Wt = np.ascontiguousarray(Wt, np.float32)
    of, inf_ = Wt.shape
    nk = inf_ // P
    a = Wt.T.reshape(nk, P, of)
    return np.ascontiguousarray(a.transpose(1, 0, 2).reshape(P, nk * of))


def _bcast(vec):
    """(H,) -> (128, HC, BC): [p, hc, b] = vec[hc*128+p]."""
    a = np.asarray(vec, np.float32).reshape(HC, P).T
    return np.ascontiguousarray(
        np.repeat(a[:, :, None], BC, axis=2).reshape(P, HC, BC))


def _perH(vec):
    """(F,) -> (128, F//128): [p, c] = vec[c*128+p]."""
    v = np.asarray(vec, np.float32)
    return np.ascontiguousarray(v.reshape(v.size // P, P).T)


def _xT(x):
    """(Bc, Tn, F) -> (128, F//128, Tn*Bc): [p, kc, t*Bc+b] = x[b, t, kc*128+p]."""
    Bc, Tn, F = x.shape
    nk = F // P
    a = x.transpose(2, 1, 0).reshape(nk, P, Tn, Bc)
    return np.ascontiguousarray(
        a.transpose(1, 0, 2, 3).reshape(P, nk, Tn * Bc).astype(np.float32))


def _fold3(Wt, bias, ln_w, ln_b):
    """Fold input-LN affine into weight/bias; return (W', bias', rowsum(W'))."""
    Wt = np.asarray(Wt, np.float32)
    Wp = Wt * np.asarray(ln_w, np.float32)[None, :]
    bp = np.asarray(bias, np.float32) + Wt @ np.asarray(ln_b, np.float32)
    return Wp, bp, Wp.sum(axis=1)


_SUBCLIP = None


def _register_custom_dve():
    """Fused out = clip(in0 - in1, s0, s1) DVE op."""
    global _SUBCLIP
    if _SUBCLIP is not None:
        return _SUBCLIP
    from concourse.dve_spec import Spec, lower, minn, maxx, Src0, Src1, C0, C1
    from concourse.dve_uop import DveOpSpec
    from concourse import dve_ops
    for o in dve_ops.OPS:
        if o.name == "SUB_CLIP_ANT":
            _SUBCLIP = o
            return o
    spec = Spec(
        body=minn(maxx(Src0 - Src1, C0), C1),
        reference=lambda in0, in1, s0, s1, imm2: np.clip(
            in0.astype(np.float32) - in1, s0, s1).astype(np.float32),
    )
    row = dve_ops._CUSTOM_DVE_ROW_BASE + len(dve_ops.OPS)
    dve_ops._SUB_OPCODE_FOR_NAME["SUB_CLIP_ANT"] = row
    shas = {}
    for ver in ("v3", "v4"):
        try:
            uops = lower(spec, ver=ver)
            shas[ver] = DveOpSpec(name="SUB_CLIP_ANT", opcode=row, uops=uops,
                                  rd1_en=True).sha(ver)
        except Exception:
            pass
    op = dve_ops.DveOp("SUB_CLIP_ANT", spec, subdim=False, uops_sha=shas)
    dve_ops.OPS.append(op)
    dve_ops.CUSTOM_DVE_SPECS[op.name] = spec
    _SUBCLIP = op
    return op


# ---------------------------------------------------------------- builder

def _build():
    import concourse.bass as bass
    import concourse.mybir as mybir
    from concourse import bacc
    from concourse.tile import TileContext
    from concourse.masks import make_identity

    f32 = mybir.dt.float32
    bf16 = mybir.dt.bfloat16
    ALU = mybir.AluOpType
    ACTF = mybir.ActivationFunctionType

    NT = T // P               # 4 time-chunks of 128 (attention)
    NBW = 512                 # proj sub-chunk column width
    NB = (CH * BC) // NBW     # 2 sub-chunks per chunk
    TB = NBW // BC            # 32 steps per sub-chunk
    NKX = IN // P             # 2

    subclip = _register_custom_dve()
    nc = bacc.Bacc("TRN2", target_bir_lowering=False)

    BF16_PARAMS = {"wrecT", "tauavT", "taubT"}
    PARAMS = [
        ("x_T", (P, NKX, TP * BC)),
        ("winT_in", (P, NKX * H)), ("tauaxT_in", (P, NKX * H)),
        ("b_iin", (P, HC)), ("b_xin", (P, HC)),
        ("wrecT", (P, HC * H)), ("tauavT", (P, HC * H)), ("taubT", (P, HC * H)),
        ("ngsbc", (P, HC, BC)), ("glbc", (P, HC, BC)), ("tbbbc", (P, HC, BC)),
        ("pAT", (P, HC * H)), ("pBT", (P, HC * H)),
        ("nrs_pA", (P, HC)), ("nrs_pB", (P, HC)),
        ("b_pA", (P, HC)), ("b_pB", (P, HC)),
        ("wqT", (P, HC * H)), ("woT", (P, HC * H)), ("wvT", (P, HC * H)),
        ("p1T", (P, HC * (H // 2))), ("p2T", (P, 2 * OUT)),
        ("b_q", (P, HC)), ("b_o", (P, HC)), ("b_p1", (P, 2)), ("b_p2", (P, 2)),
        ("rsv_flat", (P, H)), ("bv_flat", (P, H)),
        ("rolemask", (P, 1)), ("keep", (P, 1)),
    ]

    def par(name, shape):
        dt_ = bf16 if name in BF16_PARAMS else f32
        return nc.declare_dram_parameter(name, list(shape), dt_, isOutput=False)

    PR = {name: par(name, shape) for name, shape in PARAMS}
    out_p = nc.declare_dram_parameter("out", [BC, OUT], f32, isOutput=True)

    # Cross-context intermediates (ordered by TileContext exit barrier).
    hD = nc.dram_tensor("hD", [P, HC, TP, BC], f32)          # h1 chunks
    KB = nc.dram_tensor("KB", [P, NCH + 1, CH, HC, BC], bf16)  # K chunks
    mrd_d = nc.dram_tensor("mrd", [2, TP * BC], f32)          # LN stats m,r

    def load(pool, *names):
        out = {}
        for nm in names:
            t_ = pool.tile(list(PR[nm].shape), PR[nm].dtype, tag=nm, name=nm)
            nc.sync.dma_start(out=t_[:], in_=PR[nm][:])
            out[nm] = t_
        return out

    def mmT(ps, w_sb, rhs, nk, hcs=HC, wof=H):
        for hc in range(hcs):
            for kc in range(nk):
                nc.tensor.matmul(
                    ps[:, hc],
                    w_sb[:, kc * wof + hc * P: kc * wof + hc * P + P],
                    rhs[:, kc],
                    start=(kc == 0), stop=(kc == nk - 1))

    # ==================== CONTEXT 1: P1 + pipelined dual-role scan ===========
    with TileContext(nc) as tc:
        with tc.tile_pool(name="c1dram", bufs=1, space="DRAM") as dp, \
             tc.tile_pool(name="c1state", bufs=1) as sp:
            # [p, (chunk, half), hc, t, b] halves: 0 = I, 1 = X
            P1O = dp.tile([P, (NCH + 1) * 2, HC, CH, BC], f32, name="P1O")
            # pieces: NB sub-blocks of TB=CH//NB steps, shipped separately
            NBP = 2
            TBP = CH // NBP
            # hc-major so each staging DMA writes one contiguous block
            SEND = dp.tile([NCH + 1, NBP, P, 2, HC, TBP, BC], bf16, name="SEND")
            RX = dp.tile([2, NBP, 2, P, 2, HC, TBP, BC], bf16, name="RX")
            pst = sp.tile([P, HC, BC], f32, name="pst")   # pre-clip v state
            g = sp.tile([P, HC, BC], f32, name="g")
            g2 = sp.tile([P, HC, BC], f32, name="g2")
            nc.vector.memset(pst[:], 0.0)
            nc.vector.memset(g[:], 0.0)
            nc.vector.memset(g2[:], 0.0)

            # ---------------- P1: bulk input projection (x -> I0, X0) -------
            with tc.tile_pool(name="p1w", bufs=1) as p1w, \
                 tc.tile_pool(name="p1st", bufs=3) as stg1, \
                 tc.tile_pool(name="p1ps", bufs=2, space="PSUM") as pp1:
                wb = load(p1w, "winT_in", "tauaxT_in", "b_iin", "b_xin")
                xsb = p1w.tile([P, NKX, TP * BC], f32, name="xsb")
                nc.sync.dma_start(out=xsb[:], in_=PR["x_T"][:])
                for i, (wnm, bnm) in enumerate([("winT_in", "b_iin"),
                                                ("tauaxT_in", "b_xin")]):
                    for hc in range(HC):
                        for nb in range(TP * BC // NBW):
                            ps = pp1.tile([P, NBW], f32, tag="ps", name="ps")
                            for kc in range(NKX):
                                nc.tensor.matmul(
                                    ps[:],
                                    wb[wnm][:, kc * H + hc * P: kc * H + hc * P + P],
                                    xsb[:, kc, nb * NBW:(nb + 1) * NBW],
                                    start=(kc == 0), stop=(kc == NKX - 1))
                            stt = stg1.tile([P, NBW], f32, tag="st", name="stt")
                            nc.scalar.activation(stt[:], ps[:], ACTF.Identity,
                                                 bias=wb[bnm][:, hc:hc + 1])
                            c, hh = nb // 2, nb % 2
                            nc.sync.dma_start(
                                out=P1O[:, c * 2 + i, hc,
                                        hh * TB:(hh + 1) * TB, :],
                                in_=stt[:].rearrange("p (t b) -> p t b",
                                                     t=TB, b=BC))

            # ---------------- main pipelined loop ---------------------------
            with tc.tile_pool(name="scw", bufs=1) as scw, \
                 tc.tile_pool(name="cst", bufs=1) as cp:
                from concourse.masks import make_identity as _mkid
                sw = load(scw, "wrecT", "tauavT", "taubT",
                          "ngsbc", "glbc", "tbbbc",
                          "pAT", "pBT", "nrs_pA", "nrs_pB", "b_pA", "b_pB",
                          "rolemask", "keep")
                ones_col = cp.tile([P, 1], f32, name="ones_col")
                nc.vector.memset(ones_col[:], 1.0)
                ones_row = cp.tile([1, P], f32, name="ones_row")
                nc.vector.memset(ones_row[:], 1.0)
                eps_c = cp.tile([1, 1], f32, name="eps_c")
                nc.vector.memset(eps_c[:], EPS)
                zerot = cp.tile([P, HC, BC], f32, name="zerot")
                nc.vector.memset(zerot[:], 0.0)
                ident = cp.tile([P, P], f32, name="ident128")
                _mkid(nc, ident[:])
                mask = sw["rolemask"]

                with tc.tile_pool(name="scps", bufs=3, space="PSUM") as pps, \
                     tc.tile_pool(name="scwk", bufs=4) as wk, \
                     tc.tile_pool(name="scst", bufs=2) as sst, \
                     tc.tile_pool(name="rxst", bufs=1) as rxp, \
                     tc.tile_pool(name="hck", bufs=2) as hpool, \
                     tc.tile_pool(name="lsm", bufs=1) as lsm, \
                     tc.tile_pool(name="lst", bufs=1) as lst, \
                     tc.tile_pool(name="lps", bufs=2, space="PSUM") as lps, \
                     tc.tile_pool(name="lbc", bufs=1, space="PSUM") as lbc, \
                     tc.tile_pool(name="lqs", bufs=1, space="PSUM") as lqs, \
                     tc.tile_pool(name="pstg", bufs=3) as pstg:

                    def proj_chunk(k, hch, nbs):
                        """LN-fold-project pieces `nbs` of h chunk k into
                        SEND[k, nb] (+KB[k]), stats into mrd_d."""
                        for nb in nbs:
                            tl0 = nb * TB
                            hcs = [hch[:, hc, tl0:tl0 + TB, :]
                                   .rearrange("p t b -> p (t b)")
                                   for hc in range(HC)]
                            psS = lqs.tile([1, NBW], f32, tag="psSQ", name="psS")
                            for hc in range(HC):
                                nc.tensor.matmul(psS[:], ones_col[:], hcs[hc],
                                                 start=(hc == 0),
                                                 stop=(hc == HC - 1))
                            psQ = lqs.tile([1, NBW], f32, tag="psSQ", name="psQ")
                            for hc in range(HC):
                                sq = lst.tile([P, NBW], f32, tag="sq", name="sq")
                                nc.scalar.activation(sq[:], hcs[hc], ACTF.Square)
                                nc.tensor.matmul(psQ[:], ones_col[:], sq[:],
                                                 start=(hc == 0),
                                                 stop=(hc == HC - 1))
                            m_ = lsm.tile([1, NBW], f32, tag="m_", name="m_")[:]
                            r_ = lsm.tile([1, NBW], f32, tag="r_", name="r_")[:]
                            nc.scalar.activation(m_, psS[:], ACTF.Copy,
                                                 scale=1.0 / H)
                            msq = lsm.tile([1, NBW], f32, tag="msq", name="msq")
                            nc.scalar.activation(msq[:], psQ[:], ACTF.Copy,
                                                 scale=1.0 / H)
                            mm_ = lsm.tile([1, NBW], f32, tag="mm_", name="mm_")
                            nc.vector.tensor_mul(mm_[:], m_, m_)
                            var = lsm.tile([1, NBW], f32, tag="var", name="var")
                            nc.vector.tensor_sub(var[:], msq[:], mm_[:])
                            std = lsm.tile([1, NBW], f32, tag="std", name="std")
                            nc.scalar.activation(std[:], var[:], ACTF.Sqrt,
                                                 bias=eps_c[:])
                            nc.vector.reciprocal_approx_fast(out=r_, in_=std[:])
                            psM = lbc.tile([P, NBW], f32, tag="psMR", name="psM")
                            nc.tensor.matmul(psM[:], ones_row[:], m_,
                                             start=True, stop=True)
                            mB = lst.tile([P, NBW], f32, tag="mB", name="mB")
                            nc.scalar.copy(mB[:], psM[:])
                            psR = lbc.tile([P, NBW], f32, tag="psMR", name="psR")
                            nc.tensor.matmul(psR[:], ones_row[:], r_,
                                             start=True, stop=True)
                            rB = lst.tile([P, NBW], f32, tag="rB", name="rB")
                            nc.scalar.copy(rB[:], psR[:])
                            for ti, (wnm, nnm, bnm) in enumerate(
                                    [("pAT", "nrs_pA", "b_pA"),
                                     ("pBT", "nrs_pB", "b_pB")]):
                                for hc in range(HC):
                                    psP = lps.tile([P, NBW], f32, tag="psP",
                                                   name="psP")
                                    for kc in range(HC):
                                        nc.tensor.matmul(
                                            psP[:],
                                            sw[wnm][:, kc * H + hc * P:
                                                    kc * H + hc * P + P],
                                            hcs[kc],
                                            start=(kc == 0), stop=(kc == HC - 1))
                                    t2 = lst.tile([P, NBW], f32, tag="t2",
                                                  name="t2")
                                    nc.vector.scalar_tensor_tensor(
                                        t2[:], mB[:], sw[nnm][:, hc:hc + 1],
                                        psP[:], op0=ALU.mult, op1=ALU.add)
                                    f_ = lst.tile([P, NBW], f32, tag="f_",
                                                  name="f_")
                                    nc.vector.tensor_mul(f_[:], t2[:], rB[:])
                                    stt = pstg.tile([P, NBW], bf16, tag="stg",
                                                    name="stt")
                                    nc.scalar.activation(
                                        stt[:], f_[:], ACTF.Identity,
                                        bias=sw[bnm][:, hc:hc + 1])
                                    sr = stt[:].rearrange("p (t b) -> p t b",
                                                          t=TB, b=BC)
                                    nc.sync.dma_start(
                                        out=SEND[k, nb, :, ti, hc, :, :],
                                        in_=sr)
                                    if ti == 0:
                                        nc.sync.dma_start(
                                            out=KB[:, k, tl0:tl0 + TB, hc, :],
                                            in_=sr)
                            off = k * CH * BC + nb * NBW
                            nc.sync.dma_start(out=mrd_d[0:1, off:off + NBW],
                                              in_=m_)
                            nc.sync.dma_start(out=mrd_d[1:2, off:off + NBW],
                                              in_=r_)

                    def mm_group(ps, w_sb, rhs, bias_rhs):
                        """ps = ident @ bias_rhs + W @ rhs (one accum group).
                        The ident matmul injects the additive term off the
                        critical path; activations then read PSUM directly."""
                        nc.tensor.matmul(ps[:], ident[:], bias_rhs,
                                         start=True, stop=False)
                        for hc in range(HC):
                            for kc in range(HC):
                                nc.tensor.matmul(
                                    ps[:, hc],
                                    w_sb[:, kc * H + hc * P:
                                         kc * H + hc * P + P],
                                    rhs[:, kc],
                                    start=False,
                                    stop=(hc == HC - 1 and kc == HC - 1))

                    hchunk = None
                    for k in range(NCH + 1):
                        prev_h, hchunk = hchunk, hpool.tile(
                            [P, HC, CH, BC], f32, tag="hch", name="hch")
                        # ---- chunk input: local P1 + masked remote per piece
                        ich = sst.tile([P, HC, CH, BC], f32, tag="ich",
                                       name="ich")
                        xch = sst.tile([P, HC, CH, BC], f32, tag="xch",
                                       name="xch")
                        nc.sync.dma_start(out=ich[:], in_=P1O[:, 2 * k])
                        nc.sync.dma_start(out=xch[:], in_=P1O[:, 2 * k + 1])
                        if k >= 1:
                            # piece 1 of chunk k-1: project + ship
                            proj_chunk(k - 1, prev_h, [1])
                            if k - 1 < NCH:
                                nc.gpsimd.collective_compute(
                                    "AllGather", ALU.bypass,
                                    replica_groups=GROUPS,
                                    ins=[SEND[k - 1, 1].opt()],
                                    outs=[RX[(k - 1) % 2, 1].opt()])
                        for half in range(NBP):
                            t0_, t1_ = half * TBP, (half + 1) * TBP
                            if k >= 1:
                                tI = rxp.tile([P, HC, TBP, BC], bf16,
                                              tag="tI", name="tI")
                                nc.sync.dma_start(
                                    out=tI[:],
                                    in_=RX[(k - 1) % 2, half, 0, :, 0])
                                nc.vector.scalar_tensor_tensor(
                                    ich[:, :, t0_:t1_, :],
                                    tI[:], mask[:, 0:1],
                                    ich[:, :, t0_:t1_, :],
                                    op0=ALU.mult, op1=ALU.add)
                                tX = rxp.tile([P, HC, TBP, BC], bf16,
                                              tag="tI", name="tX")
                                nc.sync.dma_start(
                                    out=tX[:],
                                    in_=RX[(k - 1) % 2, half, 0, :, 1])
                                nc.vector.scalar_tensor_tensor(
                                    xch[:, :, t0_:t1_, :],
                                    tX[:], mask[:, 0:1],
                                    xch[:, :, t0_:t1_, :],
                                    op0=ALU.mult, op1=ALU.add)
                            # ---- scan steps of this half ----
                            for tl in range(t0_, t1_):
                                I0t = ich[:, :, tl, :]
                                X0t = xch[:, :, tl, :]
                                # step-start: th0 / clipped v / bf16 v
                                th = wk.tile([P, HC, BC], bf16, tag="th",
                                             name="th")
                                nc.scalar.activation(th[:], pst[:], ACTF.Tanh)
                                if tl > 0:
                                    nc.gpsimd.tensor_copy(
                                        hchunk[:, :, tl - 1, :], th[:])
                                v0 = wk.tile([P, HC, BC], f32, tag="v0",
                                             name="v0")
                                nc.vector._custom_dve(subclip, out=v0[:],
                                                      in0=pst[:], in1=zerot[:],
                                                      s0=-5.0, s1=5.0)
                                vb = wk.tile([P, HC, BC], bf16, tag="vb",
                                             name="vb")
                                nc.vector.tensor_copy(vb[:], v0[:])
                                # PE: psA, unfold-0 group, psB
                                psA = pps.tile([P, HC, BC], f32, tag="ps",
                                               name="psA")
                                mm_group(psA, sw["tauavT"], vb, X0t)
                                psI = pps.tile([P, HC, BC], f32, tag="ps",
                                               name="psI0")
                                mm_group(psI, sw["wrecT"], th, I0t)
                                th2 = wk.tile([P, HC, BC], bf16, tag="th2",
                                              name="th2")
                                nc.scalar.activation(th2[:], psA[:], ACTF.Tanh)
                                psB = pps.tile([P, HC, BC], f32, tag="ps",
                                               name="psB")
                                mm_group(psB, sw["taubT"], th2, sw["tbbbc"][:])
                                sigt = wk.tile([P, HC, BC], f32, tag="sigt",
                                               name="sigt")
                                nc.scalar.activation(sigt[:], psB[:],
                                                     ACTF.Sigmoid)
                                tau = wk.tile([P, HC, BC], f32, tag="tau",
                                              name="tau")
                                nc.vector.tensor_scalar(tau[:], sigt[:],
                                                        MAX_TAU - MIN_TAU,
                                                        MIN_TAU,
                                                        op0=ALU.mult,
                                                        op1=ALU.add)
                                rtau = wk.tile([P, HC, BC], f32, tag="rtau",
                                               name="rtau")
                                nc.vector.reciprocal_approx_fast(out=rtau[:],
                                                                 in_=tau[:])
                                a_ = wk.tile([P, HC, BC], f32, tag="a_",
                                             name="a_")
                                nc.vector.tensor_scalar(a_[:], rtau[:],
                                                        DT / 0.5, None,
                                                        op0=ALU.mult)
                                c1 = wk.tile([P, HC, BC], f32, tag="c1",
                                             name="c1")
                                nc.vector.tensor_scalar(c1[:], rtau[:],
                                                        -DT / 0.5,
                                                        1.0, op0=ALU.mult,
                                                        op1=ALU.add)
                                negam = wk.tile([P, HC, BC], f32, tag="negam",
                                                name="negam")
                                nc.vector.tensor_mul(negam[:], rtau[:],
                                                     sw["ngsbc"][:])
                                cLt = wk.tile([P, HC, BC], f32, tag="cLt",
                                              name="cLt")
                                nc.gpsimd.tensor_mul(cLt[:], rtau[:],
                                                     sw["glbc"][:])
                                cL = wk.tile([P, HC, BC], f32, tag="cL",
                                             name="cL")
                                nc.gpsimd.tensor_scalar(cL[:], cLt[:], -1.0,
                                                        1.0, op0=ALU.mult,
                                                        op1=ALU.add)
                                # ---- unfolds:
                                #   g' = (1-a)g + a*s ; p = cL*v + gam*(1-v)g'
                                gcur, gnext = g, g2
                                vcur = v0
                                for u in range(UNFOLDS):
                                    m1 = wk.tile([P, HC, BC], f32, tag="m1",
                                                 name="m1")
                                    nc.vector.tensor_mul(m1[:], c1[:],
                                                         gcur[:])
                                    w_ = wk.tile([P, HC, BC], f32, tag="w_",
                                                 name="w_")
                                    nc.gpsimd.tensor_mul(w_[:], cL[:],
                                                         vcur[:])
                                    omv = wk.tile([P, HC, BC], f32, tag="omv",
                                                  name="omv")
                                    nc.vector.tensor_scalar(omv[:], vcur[:],
                                                            -1.0, 1.0,
                                                            op0=ALU.mult,
                                                            op1=ALU.add)
                                    r_ = wk.tile([P, HC, BC], f32, tag="r_",
                                                 name="r_")
                                    nc.vector.tensor_mul(r_[:], negam[:],
                                                         omv[:])
                                    s_ = wk.tile([P, HC, BC], f32, tag="s_",
                                                 name="s_")
                                    nc.scalar.activation(s_[:], psI[:],
                                                         ACTF.Sigmoid)
                                    u_ = wk.tile([P, HC, BC], f32, tag="u_",
                                                 name="u_")
                                    nc.vector.tensor_mul(u_[:], a_[:], s_[:])
                                    nc.vector.tensor_add(gnext[:], m1[:],
                                                         u_[:])
                                    t_ = wk.tile([P, HC, BC], f32, tag="t_",
                                                 name="t_")
                                    nc.vector.tensor_mul(t_[:], gnext[:],
                                                         r_[:])
                                    # r_ = -gam*(1-v)  =>  p = w_ - t_
                                    nc.vector.tensor_sub(pst[:], w_[:], t_[:])
                                    gcur, gnext = gnext, gcur
                                    if u < UNFOLDS - 1:
                                        th_u = wk.tile([P, HC, BC], bf16,
                                                       tag="th", name="thu")
                                        nc.scalar.activation(th_u[:], pst[:],
                                                             ACTF.Tanh)
                                        psI = pps.tile([P, HC, BC], f32,
                                                       tag="ps", name="psI")
                                        mm_group(psI, sw["wrecT"], th_u, I0t)
                                        vnew = wk.tile([P, HC, BC], f32,
                                                       tag="v0", name="vnew")
                                        nc.vector._custom_dve(
                                            subclip, out=vnew[:], in0=pst[:],
                                            in1=zerot[:], s0=-5.0, s1=5.0)
                                        vcur = vnew
                                # piece 0 of chunk k complete after first
                                # step of half 1 wrote hchunk[TBP-1]
                                if tl == TBP:
                                    if k >= 1:
                                        # prev chunk's h to DRAM (off the
                                        # boundary-critical sync-queue path)
                                        nc.sync.dma_start(
                                            out=hD[:, :,
                                                   (k - 1) * CH:k * CH, :],
                                            in_=prev_h[:])
                                    proj_chunk(k, hchunk, [0])
                                    if k < NCH:
                                        nc.gpsimd.collective_compute(
                                            "AllGather", ALU.bypass,
                                            replica_groups=GROUPS,
                                            ins=[SEND[k, 0].opt()],
                                            outs=[RX[k % 2, 0].opt()])
                        # last h of chunk k = tanh(state) at chunk end
                        nc.scalar.activation(hchunk[:, :, CH - 1, :], pst[:],
                                             ACTF.Tanh)
                        if k == 0:
                            # odd cores scanned the zero warmup chunk: reset
                            nc.vector.tensor_scalar(pst[:], pst[:],
                                                    sw["keep"][:, 0:1], None,
                                                    op0=ALU.mult)
                            nc.vector.tensor_scalar(g[:], g[:],
                                                    sw["keep"][:, 0:1], None,
                                                    op0=ALU.mult)
                    # drain: final chunk's piece 1 + hD dump
                    proj_chunk(NCH, hchunk, [1])
                    nc.sync.dma_start(
                        out=hD[:, :, NCH * CH:TP, :], in_=hchunk[:])

    # ==================== CONTEXT 2: attention + head (odd cores) ============
    with TileContext(nc) as tc:
        with tc.tile_pool(name="atw", bufs=1) as atw, \
             tc.tile_pool(name="c3const", bufs=1) as cp3:
            ones_row = cp3.tile([1, P], f32, name="ones_row")
            nc.vector.memset(ones_row[:], 1.0)
            ident = cp3.tile([NH * BC, NH * BC], f32, name="ident")
            make_identity(nc, ident[:])

            wa = load(atw, "wqT", "woT", "wvT", "p1T", "p2T",
                      "b_q", "b_o", "b_p1", "b_p2", "rsv_flat", "bv_flat")

            with tc.tile_pool(name="vstats", bufs=1) as vsp:
                m1T = vsp.tile([P, NT, BC], f32, name="m1T")
                r1T = vsp.tile([P, NT, BC], f32, name="r1T")
                nc.sync.dma_start(
                    out=m1T[:],
                    in_=mrd_d[0:1, CH * BC:].rearrange(
                        "o (tc p b) -> (o p) tc b", tc=NT, p=P, b=BC))
                nc.sync.dma_start(
                    out=r1T[:],
                    in_=mrd_d[1:2, CH * BC:].rearrange(
                        "o (tc p b) -> (o p) tc b", tc=NT, p=P, b=BC))

                # ---- q at the last timestep ----
                qT = vsp.tile([P, HC, BC], bf16, name="qT")
                with tc.tile_pool(name="qps", bufs=1, space="PSUM") as qps, \
                     tc.tile_pool(name="qwk", bufs=2) as qwk, \
                     tc.tile_pool(name="qc", bufs=1) as qc:
                    hL = qc.tile([P, HC, BC], f32, name="hL")
                    nc.sync.dma_start(out=hL[:], in_=hD[:, :, TP - 1, :])
                    mlast = qc.tile([1, BC], f32, name="mlast")
                    rlast = qc.tile([1, BC], f32, name="rlast")
                    nc.sync.dma_start(
                        out=mlast[:], in_=mrd_d[0:1, (TP - 1) * BC: TP * BC])
                    nc.sync.dma_start(
                        out=rlast[:], in_=mrd_d[1:2, (TP - 1) * BC: TP * BC])
                    psb = qps.tile([P, BC], f32, tag="psb", name="psb")
                    nc.tensor.matmul(psb[:], ones_row[:], mlast[:],
                                     start=True, stop=True)
                    mL = qc.tile([P, BC], f32, name="mL")
                    nc.scalar.copy(mL[:], psb[:])
                    psb2 = qps.tile([P, BC], f32, tag="psb2", name="psb2")
                    nc.tensor.matmul(psb2[:], ones_row[:], rlast[:],
                                     start=True, stop=True)
                    rL = qc.tile([P, BC], f32, name="rL")
                    nc.scalar.copy(rL[:], psb2[:])
                    xh = qc.tile([P, HC, BC], f32, name="xh")
                    for hc in range(HC):
                        tt = qwk.tile([P, BC], f32, tag="xh1", name="tt")
                        nc.vector.tensor_sub(tt[:], hL[:, hc], mL[:])
                        nc.vector.tensor_mul(xh[:, hc], tt[:], rL[:])
                    psq = qps.tile([P, HC, BC], f32, tag="psq", name="psq")
                    mmT(psq, wa["wqT"], xh, HC)
                    for hc in range(HC):
                        nc.scalar.activation(qT[:, hc], psq[:, hc],
                                             ACTF.Identity,
                                             bias=wa["b_q"][:, hc:hc + 1])

                # ---- scores + softmax (per head-chunk K loads) ----
                sc = vsp.tile([NH * BC, T], f32, name="sc")
                with tc.tile_pool(name="scops", bufs=2, space="PSUM") as sps, \
                     tc.tile_pool(name="scowk", bufs=3) as swk, \
                     tc.tile_pool(name="ktp", bufs=2) as ktp:
                    for h in range(NH):
                        KTh = ktp.tile([P, NCH, CH, BC], bf16, tag="KTh",
                                       name="KTh")
                        nc.sync.dma_start(out=KTh[:], in_=KB[:, 1:NCH + 1, :, h, :])
                        for b in range(BC):
                            pss = sps.tile([1, T], f32, tag="pss", name="pss")
                            nc.tensor.matmul(
                                pss[:], qT[:, h, b:b + 1], KTh[:, :, :, b],
                                start=True, stop=True)
                            srow = swk.tile([1, T], f32, tag="srow",
                                            name="srow")
                            nc.scalar.copy(srow[:], pss[:])
                            nc.sync.dma_start(
                                out=sc[b * NH + h: b * NH + h + 1, :],
                                in_=srow[:])
                mx = vsp.tile([NH * BC, 1], f32, name="mx")
                nc.vector.tensor_reduce(mx[:], sc[:], axis=mybir.AxisListType.X,
                                        op=ALU.max)
                nmx = vsp.tile([NH * BC, 1], f32, name="nmx")
                nc.vector.tensor_scalar(nmx[:], mx[:], -1.0, None, op0=ALU.mult)
                ex = vsp.tile([NH * BC, T], f32, name="ex")
                sm = vsp.tile([NH * BC, 1], f32, name="sm")
                nc.scalar.activation(ex[:], sc[:], ACTF.Exp, bias=nmx[:],
                                     accum_out=sm[:])
                rsm = vsp.tile([NH * BC, 1], f32, name="rsm")
                nc.vector.reciprocal_approx_fast(out=rsm[:], in_=sm[:])
                en = vsp.tile([NH * BC, T], f32, name="en")
                nc.vector.tensor_scalar(en[:], ex[:], rsm[:], None,
                                        op0=ALU.mult)
                eT = []
                with tc.tile_pool(name="etps", bufs=2, space="PSUM") as eps_p:
                    for kc in range(NT):
                        pse = eps_p.tile([P, NH * BC], f32, tag="pse",
                                         name="pse")
                        nc.tensor.transpose(pse[:], en[:, kc * P:(kc + 1) * P],
                                            ident[:])
                        esb = vsp.tile([P, NH * BC], f32, name=f"eT{kc}",
                                       tag=f"eT{kc}")
                        nc.scalar.copy(esb[:], pse[:])
                        eT.append(esb)

                # ---- V (recomputed per example) + o ----
                psO_sb = vsp.tile([P, HC, BC], f32, name="psO_sb")
                with tc.tile_pool(name="vps", bufs=2, space="PSUM") as vps, \
                     tc.tile_pool(name="vwk", bufs=2) as vwk, \
                     tc.tile_pool(name="hbp", bufs=1) as hbp, \
                     tc.tile_pool(name="ops", bufs=1, space="PSUM") as ops_:
                    psO = ops_.tile([P, HC, BC], f32, tag="psO", name="psO")
                    BH = BC // 2
                    for bh in range(2):
                        # bulk-contiguous load of h1 for 8 examples
                        hball = hbp.tile([P, HC, T, BH], f32, tag="hball",
                                         name="hball")
                        nc.sync.dma_start(
                            out=hball[:],
                            in_=hD[:, :, CH:TP, bh * BH:(bh + 1) * BH])
                        for b8 in range(BH):
                            b = bh * BH + b8
                            Vb = vwk.tile([P, NT * H], f32, tag="Vb",
                                          name="Vb")
                            for tcc in range(NT):
                                psV = vps.tile([P, H], f32, tag="psV",
                                               name="psV")
                                for kc in range(HC):
                                    nc.tensor.matmul(
                                        psV[:],
                                        hball[:, kc, tcc * P:(tcc + 1) * P,
                                              b8],
                                        wa["wvT"][:, kc * H:(kc + 1) * H],
                                        start=(kc == 0), stop=(kc == HC - 1))
                                t2 = vwk.tile([P, H], f32, tag="t2v",
                                              name="t2")
                                nc.vector.scalar_tensor_tensor(
                                    t2[:], wa["rsv_flat"][:],
                                    m1T[:, tcc, b:b + 1],
                                    psV[:], op0=ALU.mult, op1=ALU.add)
                                f_ = vwk.tile([P, H], f32, tag="f_v",
                                              name="f_")
                                nc.vector.tensor_scalar(
                                    f_[:], t2[:], r1T[:, tcc, b:b + 1], None,
                                    op0=ALU.mult)
                                nc.vector.tensor_add(
                                    Vb[:, tcc * H:(tcc + 1) * H], f_[:],
                                    wa["bv_flat"][:])
                            for h in range(NH):
                                for kc in range(NT):
                                    nc.tensor.matmul(
                                        psO[:, h, b:b + 1],
                                        Vb[:, kc * H + h * HD:
                                           kc * H + (h + 1) * HD],
                                        eT[kc][:, b * NH + h: b * NH + h + 1],
                                        start=(kc == 0), stop=(kc == NT - 1))
                    nc.scalar.copy(psO_sb[:], psO[:])

                # ---- head ----
                with tc.tile_pool(name="hps", bufs=1, space="PSUM") as hps, \
                     tc.tile_pool(name="hc_", bufs=1) as hcp:
                    psAO = hps.tile([P, HC, BC], f32, tag="psAO", name="psAO")
                    mmT(psAO, wa["woT"], psO_sb, HC)
                    ao = hcp.tile([P, HC, BC], f32, name="ao")
                    for hc in range(HC):
                        nc.scalar.activation(ao[:, hc], psAO[:, hc],
                                             ACTF.Identity,
                                             bias=wa["b_o"][:, hc:hc + 1])
                    psP1 = hps.tile([P, 2, BC], f32, tag="psP1", name="psP1")
                    for hc in range(2):
                        for kc in range(HC):
                            nc.tensor.matmul(
                                psP1[:, hc],
                                wa["p1T"][:, kc * (H // 2) + hc * P:
                                          kc * (H // 2) + hc * P + P],
                                ao[:, kc],
                                start=(kc == 0), stop=(kc == HC - 1))
                    h1_ = hcp.tile([P, 2, BC], f32, name="h1_")
                    for hc in range(2):
                        nc.scalar.activation(h1_[:, hc], psP1[:, hc], ACTF.Relu,
                                             bias=wa["b_p1"][:, hc:hc + 1])
                    psP2 = hps.tile([P, 2, BC], f32, tag="psP2", name="psP2")
                    for hc in range(2):
                        for kc in range(2):
                            nc.tensor.matmul(
                                psP2[:, hc],
                                wa["p2T"][:, kc * OUT + hc * P:
                                          kc * OUT + hc * P + P],
                                h1_[:, kc],
                                start=(kc == 0), stop=(kc == 1))
                    outT = hcp.tile([P, 2, BC], f32, name="outT")
                    for hc in range(2):
                        nc.scalar.activation(outT[:, hc], psP2[:, hc],
                                             ACTF.Identity,
                                             bias=wa["b_p2"][:, hc:hc + 1])
                    for c in range(2):
                        nc.sync.dma_start(
                            out=out_p[:, c * P:(c + 1) * P].rearrange(
                                "b p -> p b"),
                            in_=outT[:, c])

    nc.finalize()
    return nc


# ---------------------------------------------------------------- host driver

def _prep_inputs(inputs):
    d = {k: np.asarray(v, np.float32) for k, v in inputs.items()}
    sqh = np.float32(1.0 / np.sqrt(HD))

    wi1, bi1, rs_i1 = _fold3(d["Win1_w"], d["Win1_b"], d["ln0_w"], d["ln0_b"])
    wx1, bx1, rs_x1 = _fold3(d["tau1a_w"][:, :H], d["tau1a_b"],
                             d["ln0_w"], d["ln0_b"])
    ab = d["attn_in_b"]
    wq, bq, _ = _fold3(d["attn_in_w"][0:H] * sqh, ab[0:H] * sqh,
                       d["ln1_w"], d["ln1_b"])
    wk, bk, rs_k = _fold3(d["attn_in_w"][H:2 * H], ab[H:2 * H],
                          d["ln1_w"], d["ln1_b"])
    wv, bv, rs_v = _fold3(d["attn_in_w"][2 * H:3 * H], ab[2 * H:3 * H],
                          d["ln1_w"], d["ln1_b"])

    import ml_dtypes

    def zl(shape):
        return np.zeros(shape, np.float32)

    SH = {name: shape for name, shape in [
        ("x_T", (P, IN // P, TP * BC)),
        ("winT_in", (P, (IN // P) * H)), ("tauaxT_in", (P, (IN // P) * H)),
        ("b_iin", (P, HC)), ("b_xin", (P, HC)),
        ("pAT", (P, HC * H)), ("pBT", (P, HC * H)),
        ("nrs_pA", (P, HC)), ("nrs_pB", (P, HC)),
        ("b_pA", (P, HC)), ("b_pB", (P, HC)),
        ("wqT", (P, HC * H)), ("woT", (P, HC * H)), ("wvT", (P, HC * H)),
        ("p1T", (P, HC * (H // 2))), ("p2T", (P, 2 * OUT)),
        ("b_q", (P, HC)), ("b_o", (P, HC)), ("b_p1", (P, 2)), ("b_p2", (P, 2)),
        ("rsv_flat", (P, H)), ("bv_flat", (P, H)),
    ]}

    # ---- A role (even cores): layer-0 scan + (I1, X1) projections ----
    mapA = {
        "winT_in": _wT(d["Win0_w"]), "tauaxT_in": _wT(d["tau0a_w"][:, :IN]),
        "b_iin": _perH(d["Win0_b"]), "b_xin": _perH(d["tau0a_b"]),
        "wrecT": _wT(d["Wrec0_w"]), "tauavT": _wT(d["tau0a_w"][:, IN:]),
        "taubT": _wT(d["tau0b_w"]),
        "ngsbc": _bcast(-DT * d["gsyn0"]), "glbc": _bcast(DT * d["gleak0"]),
        "tbbbc": _bcast(d["tau0b_b"]),
        "pAT": _wT(wi1), "nrs_pA": _perH(-rs_i1), "b_pA": _perH(bi1),
        "pBT": _wT(wx1), "nrs_pB": _perH(-rs_x1), "b_pB": _perH(bx1),
        "wqT": zl(SH["wqT"]), "woT": zl(SH["woT"]), "wvT": zl(SH["wvT"]),
        "p1T": zl(SH["p1T"]), "p2T": zl(SH["p2T"]),
        "b_q": zl(SH["b_q"]), "b_o": zl(SH["b_o"]),
        "b_p1": zl(SH["b_p1"]), "b_p2": zl(SH["b_p2"]),
        "rsv_flat": zl(SH["rsv_flat"]), "bv_flat": zl(SH["bv_flat"]),
        "rolemask": np.zeros((P, 1), np.float32),
        "keep": np.ones((P, 1), np.float32),
    }
    # ---- B role (odd cores): layer-1 scan + K proj + attention ----
    mapB = {
        "winT_in": zl(SH["winT_in"]), "tauaxT_in": zl(SH["tauaxT_in"]),
        "b_iin": zl(SH["b_iin"]), "b_xin": zl(SH["b_xin"]),
        "wrecT": _wT(d["Wrec1_w"]), "tauavT": _wT(d["tau1a_w"][:, H:]),
        "taubT": _wT(d["tau1b_w"]),
        "ngsbc": _bcast(-DT * d["gsyn1"]), "glbc": _bcast(DT * d["gleak1"]),
        "tbbbc": _bcast(d["tau1b_b"]),
        "pAT": _wT(wk), "nrs_pA": _perH(-rs_k), "b_pA": _perH(bk),
        "pBT": zl(SH["pBT"]), "nrs_pB": zl(SH["nrs_pB"]),
        "b_pB": zl(SH["b_pB"]),
        "wqT": _wT(wq), "b_q": _perH(bq),
        "woT": _wT(d["attn_out_w"]), "b_o": _perH(d["attn_out_b"]),
        "wvT": _wT(wv),
        "rsv_flat": np.ascontiguousarray(
            np.broadcast_to((-rs_v)[None, :], (P, H))).astype(np.float32),
        "bv_flat": np.ascontiguousarray(
            np.broadcast_to(bv[None, :], (P, H))).astype(np.float32),
        "p1T": _wT(d["p1_w"]), "b_p1": _perH(d["p1_b"]),
        "p2T": _wT(d["p2_w"]), "b_p2": _perH(d["p2_b"]),
        "rolemask": np.ones((P, 1), np.float32),
        "keep": np.zeros((P, 1), np.float32),
    }
    for m in (mapA, mapB):
        for nm in ("wrecT", "tauavT", "taubT"):
            m[nm] = m[nm].astype(ml_dtypes.bfloat16)

    x = d["inputs"]
    xz = np.zeros((BC, TP, IN), np.float32)
    in_maps = []
    for pr in range(NPAIR):
        xp = np.zeros((BC, TP, IN), np.float32)
        xp[:, :T] = x[pr * BC:(pr + 1) * BC]
        mA = dict(mapA)
        mA["x_T"] = _xT(xp)
        mB = dict(mapB)
        mB["x_T"] = _xT(xz)
        in_maps.append(mA)
        in_maps.append(mB)
    return in_maps


def _run(inputs, trace=False):
    from concourse.bass_utils import run_bass_kernel_spmd
    if "nc" not in _CACHE:
        _CACHE["nc"] = _build()
    nc = _CACHE["nc"]
    in_maps = _prep_inputs(inputs)
    res = run_bass_kernel_spmd(nc, in_maps, list(range(NCORES)), trace=trace)
    full = np.zeros((B, OUT), np.float32)
    for pr in range(NPAIR):
        full[pr * BC:(pr + 1) * BC] = res.results[2 * pr + 1]["out"]
    return full, res


def kernel(**inputs):
    out, _ = _run(inputs, trace=False)
    return out



# revision 29
# speedup vs baseline: 1.1863x; 1.1863x over previous
"""Trainium2 Bass kernel for nn_AdaptiveLNN — 2-stage cross-core pipeline.

The recurrent scan is weight-load bound: each layer-step issues 128
LDWEIGHTS+MATMUL pairs (8 H x H matrix passes x 16 chunks of 128x128)
whose cost is independent of the batch-column count N.  The baseline
ran both layers' scans on every core (1024 layer-steps at N=8).  Here:

  - Cores are paired {2p, 2p+1}; pair p owns examples [16p, 16p+16).
  - Even cores scan LAYER 0 (N=16), odd cores scan LAYER 1 + attention.
  - Per 64-step chunk, the even core LN-projects its h0 chunk into
    (I1, X1) and ships it to the odd core via a paired AllGather
    (groups [[0,1],[2,3],[4,5],[6,7]]); the odd core consumes it one
    iteration later (1-chunk pipeline skew).
  - SPMD: every core runs the SAME program; roles differ only in the
    DATA (weights, masks).  The scan input is blended as
    ich = P1_local + rolemask * RX  (rolemask: even=0, odd=1), and odd
    cores reset state after the (zero-input) warmup chunk via a
    multiplicative keep mask.
  - Each core then runs 512+64 layer-steps instead of 1024 -> ~1.8x.

Attention: only q[T-1] needed -> O(T) attention on the odd cores (K
from the per-chunk projections, V recomputed per-example from h1).
"""

import numpy as np

B, T, IN, H, OUT, NH = 64, 512, 256, 512, 256, 4
HD = H // NH
DT = 0.1
UNFOLDS = 6
MIN_TAU, MAX_TAU = 0.1, 10.0
NCORES = 8
NPAIR = 4
BC = B // NPAIR           # 16 examples per pair
HC = H // 128             # 4
P = 128
EPS = 1e-5
CH = 64                   # pipeline chunk (steps)
NCH = T // CH             # 8
TP = T + CH               # padded timeline (576) incl. warmup/drain chunk
GROUPS = [[0, 1], [2, 3], [4, 5], [6, 7]]

_CACHE = {}


# ---------------------------------------------------------------- host packing

def _wT(Wt):
    """(out_f, in_f) -> lhsT sbuf layout (128, nk*out_f):
    [p, kc*out_f + m] = W[m, kc*128 + p]."""
    Wt = np.ascontiguousarray(Wt, np.float32)
    of, inf_ = Wt.shape
    nk = inf_ // P
    a = Wt.T.reshape(nk, P, of)
    return np.ascontiguousarray(a.transpose(1, 0, 2).reshape(P, nk * of))


def _bcast(vec):
    """(H,) -> (128, HC, BC): [p, hc, b] = vec[hc*128+p]."""
    a = np.asarray(vec, np.float32).reshape(HC, P).T
    return np.ascontiguousarray(
        np.repeat(a[:, :, None], BC, axis=2).reshape(P, HC, BC))


def _perH(vec):
    """(F,) -> (128, F//128): [p, c] = vec[c*128+p]."""
    v = np.asarray(vec, np.float32)
    return np.ascontiguousarray(v.reshape(v.size // P, P).T)


def _xT(x):
    """(Bc, Tn, F) -> (128, F//128, Tn*Bc): [p, kc, t*Bc+b] = x[b, t, kc*128+p]."""
    Bc, Tn, F = x.shape
    nk = F // P
    a = x.transpose(2, 1, 0).reshape(nk, P, Tn, Bc)
    return np.ascontiguousarray(
        a.transpose(1, 0, 2, 3).reshape(P, nk, Tn * Bc).astype(np.float32))


def _fold3(Wt, bias, ln_w, ln_b):
    """Fold input-LN affine into weight/bias; return (W', bias', rowsum(W'))."""
    Wt = np.asarray(Wt, np.float32)
    Wp = Wt * np.asarray(ln_w, np.float32)[None, :]
    bp = np.asarray(bias, np.float32) + Wt @ np.asarray(ln_b, np.float32)
    return Wp, bp, Wp.sum(axis=1)


_SUBCLIP = None


def _register_custom_dve():
    """Fused out = clip(in0 - in1, s0, s1) DVE op."""
    global _SUBCLIP
    if _SUBCLIP is not None:
        return _SUBCLIP
    from concourse.dve_spec import Spec, lower, minn, maxx, Src0, Src1, C0, C1
    from concourse.dve_uop import DveOpSpec
    from concourse import dve_ops
    for o in dve_ops.OPS:
        if o.name == "SUB_CLIP_ANT":
            _SUBCLIP = o
            return o
    spec = Spec(
        body=minn(maxx(Src0 - Src1, C0), C1),
        reference=lambda in0, in1, s0, s1, imm2: np.clip(
            in0.astype(np.float32) - in1, s0, s1).astype(np.float32),
    )
    row = dve_ops._CUSTOM_DVE_ROW_BASE + len(dve_ops.OPS)
    dve_ops._SUB_OPCODE_FOR_NAME["SUB_CLIP_ANT"] = row
    shas = {}
    for ver in ("v3", "v4"):
        try:
            uops = lower(spec, ver=ver)
            shas[ver] = DveOpSpec(name="SUB_CLIP_ANT", opcode=row, uops=uops,
                                  rd1_en=True).sha(ver)
        except Exception:
            pass
    op = dve_ops.DveOp("SUB_CLIP_ANT", spec, subdim=False, uops_sha=shas)
    dve_ops.OPS.append(op)
    dve_ops.CUSTOM_DVE_SPECS[op.name] = spec
    _SUBCLIP = op
    return op


# ---------------------------------------------------------------- builder

def _build():
    import concourse.bass as bass
    import concourse.mybir as mybir
    from concourse import bacc
    from concourse.tile import TileContext
    from concourse.masks import make_identity

    f32 = mybir.dt.float32
    bf16 = mybir.dt.bfloat16
    ALU = mybir.AluOpType
    ACTF = mybir.ActivationFunctionType

    NT = T // P               # 4 time-chunks of 128 (attention)
    NBW = 512                 # proj sub-chunk column width
    NB = (CH * BC) // NBW     # 2 sub-chunks per chunk
    TB = NBW // BC            # 32 steps per sub-chunk
    NKX = IN // P             # 2

    subclip = _register_custom_dve()
    nc = bacc.Bacc("TRN2", target_bir_lowering=False)

    BF16_PARAMS = {"wrecT", "tauavT", "taubT"}
    PARAMS = [
        ("x_T", (P, NKX, TP * BC)),
        ("winT_in", (P, NKX * H)), ("tauaxT_in", (P, NKX * H)),
        ("b_iin", (P, HC)), ("b_xin", (P, HC)),
        ("wrecT", (P, HC * H)), ("tauavT", (P, HC * H)), ("taubT", (P, HC * H)),
        ("ngsbc", (P, HC, BC)), ("glbc", (P, HC, BC)), ("tbbbc", (P, HC, BC)),
        ("pAT", (P, HC * H)), ("pBT", (P, HC * H)),
        ("nrs_pA", (P, HC)), ("nrs_pB", (P, HC)),
        ("b_pA", (P, HC)), ("b_pB", (P, HC)),
        ("wqT", (P, HC * H)), ("woT", (P, HC * H)), ("wvT", (P, HC * H)),
        ("p1T", (P, HC * (H // 2))), ("p2T", (P, 2 * OUT)),
        ("b_q", (P, HC)), ("b_o", (P, HC)), ("b_p1", (P, 2)), ("b_p2", (P, 2)),
        ("rsv_flat", (P, H)), ("bv_flat", (P, H)),
        ("rolemask", (P, 1)), ("keep", (P, 1)),
    ]

    def par(name, shape):
        dt_ = bf16 if name in BF16_PARAMS else f32
        return nc.declare_dram_parameter(name, list(shape), dt_, isOutput=False)

    PR = {name: par(name, shape) for name, shape in PARAMS}
    out_p = nc.declare_dram_parameter("out", [BC, OUT], f32, isOutput=True)

    # Cross-context intermediates (ordered by TileContext exit barrier).
    hD = nc.dram_tensor("hD", [P, HC, TP, BC], f32)          # h1 chunks
    KB = nc.dram_tensor("KB", [P, NCH + 1, CH, HC, BC], bf16)  # K chunks
    mrd_d = nc.dram_tensor("mrd", [2, TP * BC], f32)          # LN stats m,r

    def load(pool, *names):
        out = {}
        for nm in names:
            t_ = pool.tile(list(PR[nm].shape), PR[nm].dtype, tag=nm, name=nm)
            nc.sync.dma_start(out=t_[:], in_=PR[nm][:])
            out[nm] = t_
        return out

    def mmT(ps, w_sb, rhs, nk, hcs=HC, wof=H):
        for hc in range(hcs):
            for kc in range(nk):
                nc.tensor.matmul(
                    ps[:, hc],
                    w_sb[:, kc * wof + hc * P: kc * wof + hc * P + P],
                    rhs[:, kc],
                    start=(kc == 0), stop=(kc == nk - 1))

    # ==================== CONTEXT 1: P1 + pipelined dual-role scan ===========
    with TileContext(nc) as tc:
        with tc.tile_pool(name="c1dram", bufs=1, space="DRAM") as dp, \
             tc.tile_pool(name="c1state", bufs=1) as sp:
            # [p, (chunk, half), hc, t, b] halves: 0 = I, 1 = X
            P1O = dp.tile([P, (NCH + 1) * 2, HC, CH, BC], f32, name="P1O")
            # pieces: NB sub-blocks of TB=CH//NB steps, shipped separately
            NBP = 2
            TBP = CH // NBP
            # hc-major so each staging DMA writes one contiguous block
            SEND = dp.tile([NCH + 1, NBP, P, 2, HC, TBP, BC], bf16, name="SEND")
            RX = dp.tile([2, NBP, 2, P, 2, HC, TBP, BC], bf16, name="RX")
            pst = sp.tile([P, HC, BC], f32, name="pst")   # pre-clip v state
            g = sp.tile([P, HC, BC], f32, name="g")
            g2 = sp.tile([P, HC, BC], f32, name="g2")
            nc.vector.memset(pst[:], 0.0)
            nc.vector.memset(g[:], 0.0)
            nc.vector.memset(g2[:], 0.0)

            # ---------------- P1: bulk input projection (x -> I0, X0) -------
            with tc.tile_pool(name="p1w", bufs=1) as p1w, \
                 tc.tile_pool(name="p1st", bufs=3) as stg1, \
                 tc.tile_pool(name="p1ps", bufs=2, space="PSUM") as pp1:
                wb = load(p1w, "winT_in", "tauaxT_in", "b_iin", "b_xin")
                xsb = p1w.tile([P, NKX, TP * BC], f32, name="xsb")
                nc.sync.dma_start(out=xsb[:], in_=PR["x_T"][:])
                for i, (wnm, bnm) in enumerate([("winT_in", "b_iin"),
                                                ("tauaxT_in", "b_xin")]):
                    for hc in range(HC):
                        for nb in range(TP * BC // NBW):
                            ps = pp1.tile([P, NBW], f32, tag="ps", name="ps")
                            for kc in range(NKX):
                                nc.tensor.matmul(
                                    ps[:],
                                    wb[wnm][:, kc * H + hc * P: kc * H + hc * P + P],
                                    xsb[:, kc, nb * NBW:(nb + 1) * NBW],
                                    start=(kc == 0), stop=(kc == NKX - 1))
                            stt = stg1.tile([P, NBW], f32, tag="st", name="stt")
                            nc.scalar.activation(stt[:], ps[:], ACTF.Identity,
                                                 bias=wb[bnm][:, hc:hc + 1])
                            c, hh = nb // 2, nb % 2
                            nc.sync.dma_start(
                                out=P1O[:, c * 2 + i, hc,
                                        hh * TB:(hh + 1) * TB, :],
                                in_=stt[:].rearrange("p (t b) -> p t b",
                                                     t=TB, b=BC))

            # ---------------- main pipelined loop ---------------------------
            with tc.tile_pool(name="scw", bufs=1) as scw, \
                 tc.tile_pool(name="cst", bufs=1) as cp:
                from concourse.masks import make_identity as _mkid
                sw = load(scw, "wrecT", "tauavT", "taubT",
                          "ngsbc", "glbc", "tbbbc",
                          "pAT", "pBT", "nrs_pA", "nrs_pB", "b_pA", "b_pB",
                          "rolemask", "keep")
                ones_col = cp.tile([P, 1], f32, name="ones_col")
                nc.vector.memset(ones_col[:], 1.0)
                ones_row = cp.tile([1, P], f32, name="ones_row")
                nc.vector.memset(ones_row[:], 1.0)
                eps_c = cp.tile([1, 1], f32, name="eps_c")
                nc.vector.memset(eps_c[:], EPS)
                zerot = cp.tile([P, HC, BC], f32, name="zerot")
                nc.vector.memset(zerot[:], 0.0)
                ident = cp.tile([P, P], f32, name="ident128")
                _mkid(nc, ident[:])
                mask = sw["rolemask"]

                with tc.tile_pool(name="scps", bufs=3, space="PSUM") as pps, \
                     tc.tile_pool(name="scwk", bufs=4) as wk, \
                     tc.tile_pool(name="scst", bufs=2) as sst, \
                     tc.tile_pool(name="rxst", bufs=1) as rxp, \
                     tc.tile_pool(name="hck", bufs=2) as hpool, \
                     tc.tile_pool(name="lsm", bufs=1) as lsm, \
                     tc.tile_pool(name="lst", bufs=1) as lst, \
                     tc.tile_pool(name="lps", bufs=2, space="PSUM") as lps, \
                     tc.tile_pool(name="lbc", bufs=1, space="PSUM") as lbc, \
                     tc.tile_pool(name="lqs", bufs=1, space="PSUM") as lqs, \
                     tc.tile_pool(name="pstg", bufs=3) as pstg:

                    def proj_chunk(k, hch, nbs):
                        """LN-fold-project pieces `nbs` of h chunk k into
                        SEND[k, nb] (+KB[k]), stats into mrd_d."""
                        for nb in nbs:
                            tl0 = nb * TB
                            hcs = [hch[:, hc, tl0:tl0 + TB, :]
                                   .rearrange("p t b -> p (t b)")
                                   for hc in range(HC)]
                            psS = lqs.tile([1, NBW], f32, tag="psSQ", name="psS")
                            for hc in range(HC):
                                nc.tensor.matmul(psS[:], ones_col[:], hcs[hc],
                                                 start=(hc == 0),
                                                 stop=(hc == HC - 1))
                            psQ = lqs.tile([1, NBW], f32, tag="psSQ", name="psQ")
                            for hc in range(HC):
                                sq = lst.tile([P, NBW], f32, tag="sq", name="sq")
                                nc.scalar.activation(sq[:], hcs[hc], ACTF.Square)
                                nc.tensor.matmul(psQ[:], ones_col[:], sq[:],
                                                 start=(hc == 0),
                                                 stop=(hc == HC - 1))
                            m_ = lsm.tile([1, NBW], f32, tag="m_", name="m_")[:]
                            r_ = lsm.tile([1, NBW], f32, tag="r_", name="r_")[:]
                            nc.scalar.activation(m_, psS[:], ACTF.Copy,
                                                 scale=1.0 / H)
                            msq = lsm.tile([1, NBW], f32, tag="msq", name="msq")
                            nc.scalar.activation(msq[:], psQ[:], ACTF.Copy,
                                                 scale=1.0 / H)
                            mm_ = lsm.tile([1, NBW], f32, tag="mm_", name="mm_")
                            nc.vector.tensor_mul(mm_[:], m_, m_)
                            var = lsm.tile([1, NBW], f32, tag="var", name="var")
                            nc.vector.tensor_sub(var[:], msq[:], mm_[:])
                            std = lsm.tile([1, NBW], f32, tag="std", name="std")
                            nc.scalar.activation(std[:], var[:], ACTF.Sqrt,
                                                 bias=eps_c[:])
                            nc.vector.reciprocal_approx_fast(out=r_, in_=std[:])
                            psM = lbc.tile([P, NBW], f32, tag="psMR", name="psM")
                            nc.tensor.matmul(psM[:], ones_row[:], m_,
                                             start=True, stop=True)
                            mB = lst.tile([P, NBW], f32, tag="mB", name="mB")
                            nc.scalar.copy(mB[:], psM[:])
                            psR = lbc.tile([P, NBW], f32, tag="psMR", name="psR")
                            nc.tensor.matmul(psR[:], ones_row[:], r_,
                                             start=True, stop=True)
                            rB = lst.tile([P, NBW], f32, tag="rB", name="rB")
                            nc.scalar.copy(rB[:], psR[:])
                            for ti, (wnm, nnm, bnm) in enumerate(
                                    [("pAT", "nrs_pA", "b_pA"),
                                     ("pBT", "nrs_pB", "b_pB")]):
                                for hc in range(HC):
                                    psP = lps.tile([P, NBW], f32, tag="psP",
                                                   name="psP")
                                    for kc in range(HC):
                                        nc.tensor.matmul(
                                            psP[:],
                                            sw[wnm][:, kc * H + hc * P:
                                                    kc * H + hc * P + P],
                                            hcs[kc],
                                            start=(kc == 0), stop=(kc == HC - 1))
                                    t2 = lst.tile([P, NBW], f32, tag="t2",
                                                  name="t2")
                                    nc.vector.scalar_tensor_tensor(
                                        t2[:], mB[:], sw[nnm][:, hc:hc + 1],
                                        psP[:], op0=ALU.mult, op1=ALU.add)
                                    f_ = lst.tile([P, NBW], f32, tag="f_",
                                                  name="f_")
                                    nc.vector.tensor_mul(f_[:], t2[:], rB[:])
                                    stt = pstg.tile([P, NBW], bf16, tag="stg",
                                                    name="stt")
                                    nc.scalar.activation(
                                        stt[:], f_[:], ACTF.Identity,
                                        bias=sw[bnm][:, hc:hc + 1])
                                    sr = stt[:].rearrange("p (t b) -> p t b",
                                                          t=TB, b=BC)
                                    nc.sync.dma_start(
                                        out=SEND[k, nb, :, ti, hc, :, :],
                                        in_=sr)
                                    if ti == 0:
                                        nc.sync.dma_start(
                                            out=KB[:, k, tl0:tl0 + TB, hc, :],
                                            in_=sr)
                            off = k * CH * BC + nb * NBW
                            nc.sync.dma_start(out=mrd_d[0:1, off:off + NBW],
                                              in_=m_)
                            nc.sync.dma_start(out=mrd_d[1:2, off:off + NBW],
                                              in_=r_)

                    def mm_group(ps, w_sb, rhs, bias_rhs):
                        """ps = ident @ bias_rhs + W @ rhs (one accum group).
                        The ident matmul injects the additive term off the
                        critical path; activations then read PSUM directly."""
                        nc.tensor.matmul(ps[:], ident[:], bias_rhs,
                                         start=True, stop=False)
                        for hc in range(HC):
                            for kc in range(HC):
                                nc.tensor.matmul(
                                    ps[:, hc],
                                    w_sb[:, kc * H + hc * P:
                                         kc * H + hc * P + P],
                                    rhs[:, kc],
                                    start=False,
                                    stop=(hc == HC - 1 and kc == HC - 1))

                    hchunk = None
                    for k in range(NCH + 1):
                        prev_h, hchunk = hchunk, hpool.tile(
                            [P, HC, CH, BC], f32, tag="hch", name="hch")
                        # ---- chunk input: local P1 + masked remote per piece
                        ich = sst.tile([P, HC, CH, BC], f32, tag="ich",
                                       name="ich")
                        xch = sst.tile([P, HC, CH, BC], f32, tag="xch",
                                       name="xch")
                        nc.sync.dma_start(out=ich[:], in_=P1O[:, 2 * k])
                        nc.sync.dma_start(out=xch[:], in_=P1O[:, 2 * k + 1])
                        if k >= 1:
                            # piece 1 of chunk k-1: project + ship
                            proj_chunk(k - 1, prev_h, [1])
                            if k - 1 < NCH:
                                nc.gpsimd.collective_compute(
                                    "AllGather", ALU.bypass,
                                    replica_groups=GROUPS,
                                    ins=[SEND[k - 1, 1].opt()],
                                    outs=[RX[(k - 1) % 2, 1].opt()])
                        for half in range(NBP):
                            t0_, t1_ = half * TBP, (half + 1) * TBP
                            if k >= 1:
                                tI = rxp.tile([P, HC, TBP, BC], bf16,
                                              tag="tI", name="tI")
                                nc.sync.dma_start(
                                    out=tI[:],
                                    in_=RX[(k - 1) % 2, half, 0, :, 0])
                                nc.vector.scalar_tensor_tensor(
                                    ich[:, :, t0_:t1_, :],
                                    tI[:], mask[:, 0:1],
                                    ich[:, :, t0_:t1_, :],
                                    op0=ALU.mult, op1=ALU.add)
                                tX = rxp.tile([P, HC, TBP, BC], bf16,
                                              tag="tI", name="tX")
                                nc.sync.dma_start(
                                    out=tX[:],
                                    in_=RX[(k - 1) % 2, half, 0, :, 1])
                                nc.vector.scalar_tensor_tensor(
                                    xch[:, :, t0_:t1_, :],
                                    tX[:], mask[:, 0:1],
                                    xch[:, :, t0_:t1_, :],
                                    op0=ALU.mult, op1=ALU.add)
                            # ---- scan steps of this half ----
                            for tl in range(t0_, t1_):
                                I0t = ich[:, :, tl, :]
                                X0t = xch[:, :, tl, :]
                                # step-start: th0 / clipped v / bf16 v
                                th = wk.tile([P, HC, BC], bf16, tag="th",
                                             name="th")
                                nc.scalar.activation(th[:], pst[:], ACTF.Tanh)
                                if tl > 0:
                                    nc.gpsimd.tensor_copy(
                                        hchunk[:, :, tl - 1, :], th[:])
                                v0 = wk.tile([P, HC, BC], f32, tag="v0",
                                             name="v0")
                                nc.vector._custom_dve(subclip, out=v0[:],
                                                      in0=pst[:], in1=zerot[:],
                                                      s0=-5.0, s1=5.0)
                                vb = wk.tile([P, HC, BC], bf16, tag="vb",
                                             name="vb")
                                nc.gpsimd.tensor_copy(vb[:], v0[:])
                                # PE: psA, unfold-0 group, psB
                                psA = pps.tile([P, HC, BC], f32, tag="ps",
                                               name="psA")
                                mm_group(psA, sw["tauavT"], vb, X0t)
                                psI = pps.tile([P, HC, BC], f32, tag="ps",
                                               name="psI0")
                                mm_group(psI, sw["wrecT"], th, I0t)
                                th2 = wk.tile([P, HC, BC], bf16, tag="th2",
                                              name="th2")
                                nc.scalar.activation(th2[:], psA[:], ACTF.Tanh)
                                psB = pps.tile([P, HC, BC], f32, tag="ps",
                                               name="psB")
                                mm_group(psB, sw["taubT"], th2, sw["tbbbc"][:])
                                sigt = wk.tile([P, HC, BC], f32, tag="sigt",
                                               name="sigt")
                                nc.scalar.activation(sigt[:], psB[:],
                                                     ACTF.Sigmoid)
                                tau = wk.tile([P, HC, BC], f32, tag="tau",
                                              name="tau")
                                nc.vector.tensor_scalar(tau[:], sigt[:],
                                                        MAX_TAU - MIN_TAU,
                                                        MIN_TAU,
                                                        op0=ALU.mult,
                                                        op1=ALU.add)
                                rtau = wk.tile([P, HC, BC], f32, tag="rtau",
                                               name="rtau")
                                nc.vector.reciprocal_approx_fast(out=rtau[:],
                                                                 in_=tau[:])
                                a_ = wk.tile([P, HC, BC], f32, tag="a_",
                                             name="a_")
                                nc.vector.tensor_scalar(a_[:], rtau[:],
                                                        DT / 0.5, None,
                                                        op0=ALU.mult)
                                c1 = wk.tile([P, HC, BC], f32, tag="c1",
                                             name="c1")
                                nc.vector.tensor_scalar(c1[:], rtau[:],
                                                        -DT / 0.5,
                                                        1.0, op0=ALU.mult,
                                                        op1=ALU.add)
                                negam = wk.tile([P, HC, BC], f32, tag="negam",
                                                name="negam")
                                nc.gpsimd.tensor_mul(negam[:], rtau[:],
                                                     sw["ngsbc"][:])
                                cLt = wk.tile([P, HC, BC], f32, tag="cLt",
                                              name="cLt")
                                nc.gpsimd.tensor_mul(cLt[:], rtau[:],
                                                     sw["glbc"][:])
                                cL = wk.tile([P, HC, BC], f32, tag="cL",
                                             name="cL")
                                nc.gpsimd.tensor_scalar(cL[:], cLt[:], -1.0,
                                                        1.0, op0=ALU.mult,
                                                        op1=ALU.add)
                                # ---- unfolds:
                                #   g' = (1-a)g + a*s ; p = cL*v + gam*(1-v)g'
                                gcur, gnext = g, g2
                                vcur = v0
                                for u in range(UNFOLDS):
                                    m1 = wk.tile([P, HC, BC], f32, tag="m1",
                                                 name="m1")
                                    nc.gpsimd.tensor_mul(m1[:], c1[:],
                                                         gcur[:])
                                    w_ = wk.tile([P, HC, BC], f32, tag="w_",
                                                 name="w_")
                                    nc.gpsimd.tensor_mul(w_[:], cL[:],
                                                         vcur[:])
                                    omv = wk.tile([P, HC, BC], f32, tag="omv",
                                                  name="omv")
                                    nc.gpsimd.tensor_scalar(omv[:], vcur[:],
                                                            -1.0, 1.0,
                                                            op0=ALU.mult,
                                                            op1=ALU.add)
                                    r_ = wk.tile([P, HC, BC], f32, tag="r_",
                                                 name="r_")
                                    nc.vector.tensor_mul(r_[:], negam[:],
                                                         omv[:])
                                    s_ = wk.tile([P, HC, BC], f32, tag="s_",
                                                 name="s_")
                                    nc.scalar.activation(s_[:], psI[:],
                                                         ACTF.Sigmoid)
                                    u_ = wk.tile([P, HC, BC], f32, tag="u_",
                                                 name="u_")
                                    nc.vector.tensor_mul(u_[:], a_[:], s_[:])
                                    nc.vector.tensor_add(gnext[:], m1[:],
                                                         u_[:])
                                    t_ = wk.tile([P, HC, BC], f32, tag="t_",
                                                 name="t_")
                                    nc.vector.tensor_mul(t_[:], gnext[:],
                                                         r_[:])
                                    # r_ = -gam*(1-v)  =>  p = w_ - t_
                                    nc.vector.tensor_sub(pst[:], w_[:], t_[:])
                                    gcur, gnext = gnext, gcur
                                    if u < UNFOLDS - 1:
                                        th_u = wk.tile([P, HC, BC], bf16,
                                                       tag="th", name="thu")
                                        nc.scalar.activation(th_u[:], pst[:],
                                                             ACTF.Tanh)
                                        psI = pps.tile([P, HC, BC], f32,
                                                       tag="ps", name="psI")
                                        mm_group(psI, sw["wrecT"], th_u, I0t)
                                        vnew = wk.tile([P, HC, BC], f32,
                                                       tag="v0", name="vnew")
                                        nc.vector._custom_dve(
                                            subclip, out=vnew[:], in0=pst[:],
                                            in1=zerot[:], s0=-5.0, s1=5.0)
                                        vcur = vnew
                                # piece 0 of chunk k complete after first
                                # step of half 1 wrote hchunk[TBP-1]
                                if tl == TBP:
                                    if k >= 1:
                                        # prev chunk's h to DRAM (off the
                                        # boundary-critical sync-queue path)
                                        nc.sync.dma_start(
                                            out=hD[:, :,
                                                   (k - 1) * CH:k * CH, :],
                                            in_=prev_h[:])
                                    proj_chunk(k, hchunk, [0])
                                    if k < NCH:
                                        nc.gpsimd.collective_compute(
                                            "AllGather", ALU.bypass,
                                            replica_groups=GROUPS,
                                            ins=[SEND[k, 0].opt()],
                                            outs=[RX[k % 2, 0].opt()])
                        # last h of chunk k = tanh(state) at chunk end
                        nc.scalar.activation(hchunk[:, :, CH - 1, :], pst[:],
                                             ACTF.Tanh)
                        if k == 0:
                            # odd cores scanned the zero warmup chunk: reset
                            nc.vector.tensor_scalar(pst[:], pst[:],
                                                    sw["keep"][:, 0:1], None,
                                                    op0=ALU.mult)
                            nc.vector.tensor_scalar(g[:], g[:],
                                                    sw["keep"][:, 0:1], None,
                                                    op0=ALU.mult)
                    # drain: final chunk's piece 1 + hD dump
                    proj_chunk(NCH, hchunk, [1])
                    nc.sync.dma_start(
                        out=hD[:, :, NCH * CH:TP, :], in_=hchunk[:])

    # ==================== CONTEXT 2: attention + head (odd cores) ============
    with TileContext(nc) as tc:
        with tc.tile_pool(name="atw", bufs=1) as atw, \
             tc.tile_pool(name="c3const", bufs=1) as cp3:
            ones_row = cp3.tile([1, P], f32, name="ones_row")
            nc.vector.memset(ones_row[:], 1.0)
            ident = cp3.tile([NH * BC, NH * BC], f32, name="ident")
            make_identity(nc, ident[:])

            wa = load(atw, "wqT", "woT", "wvT", "p1T", "p2T",
                      "b_q", "b_o", "b_p1", "b_p2", "rsv_flat", "bv_flat")

            with tc.tile_pool(name="vstats", bufs=1) as vsp:
                m1T = vsp.tile([P, NT, BC], f32, name="m1T")
                r1T = vsp.tile([P, NT, BC], f32, name="r1T")
                nc.sync.dma_start(
                    out=m1T[:],
                    in_=mrd_d[0:1, CH * BC:].rearrange(
                        "o (tc p b) -> (o p) tc b", tc=NT, p=P, b=BC))
                nc.sync.dma_start(
                    out=r1T[:],
                    in_=mrd_d[1:2, CH * BC:].rearrange(
                        "o (tc p b) -> (o p) tc b", tc=NT, p=P, b=BC))

                # ---- q at the last timestep ----
                qT = vsp.tile([P, HC, BC], bf16, name="qT")
                with tc.tile_pool(name="qps", bufs=1, space="PSUM") as qps, \
                     tc.tile_pool(name="qwk", bufs=2) as qwk, \
                     tc.tile_pool(name="qc", bufs=1) as qc:
                    hL = qc.tile([P, HC, BC], f32, name="hL")
                    nc.sync.dma_start(out=hL[:], in_=hD[:, :, TP - 1, :])
                    mlast = qc.tile([1, BC], f32, name="mlast")
                    rlast = qc.tile([1, BC], f32, name="rlast")
                    nc.sync.dma_start(
                        out=mlast[:], in_=mrd_d[0:1, (TP - 1) * BC: TP * BC])
                    nc.sync.dma_start(
                        out=rlast[:], in_=mrd_d[1:2, (TP - 1) * BC: TP * BC])
                    psb = qps.tile([P, BC], f32, tag="psb", name="psb")
                    nc.tensor.matmul(psb[:], ones_row[:], mlast[:],
                                     start=True, stop=True)
                    mL = qc.tile([P, BC], f32, name="mL")
                    nc.scalar.copy(mL[:], psb[:])
                    psb2 = qps.tile([P, BC], f32, tag="psb2", name="psb2")
                    nc.tensor.matmul(psb2[:], ones_row[:], rlast[:],
                                     start=True, stop=True)
                    rL = qc.tile([P, BC], f32, name="rL")
                    nc.scalar.copy(rL[:], psb2[:])
                    xh = qc.tile([P, HC, BC], f32, name="xh")
                    for hc in range(HC):
                        tt = qwk.tile([P, BC], f32, tag="xh1", name="tt")
                        nc.vector.tensor_sub(tt[:], hL[:, hc], mL[:])
                        nc.vector.tensor_mul(xh[:, hc], tt[:], rL[:])
                    psq = qps.tile([P, HC, BC], f32, tag="psq", name="psq")
                    mmT(psq, wa["wqT"], xh, HC)
                    for hc in range(HC):
                        nc.scalar.activation(qT[:, hc], psq[:, hc],
                                             ACTF.Identity,
                                             bias=wa["b_q"][:, hc:hc + 1])

                # ---- scores + softmax (per head-chunk K loads) ----
                sc = vsp.tile([NH * BC, T], f32, name="sc")
                with tc.tile_pool(name="scops", bufs=2, space="PSUM") as sps, \
                     tc.tile_pool(name="scowk", bufs=3) as swk, \
                     tc.tile_pool(name="ktp", bufs=2) as ktp:
                    for h in range(NH):
                        KTh = ktp.tile([P, NCH, CH, BC], bf16, tag="KTh",
                                       name="KTh")
                        nc.sync.dma_start(out=KTh[:], in_=KB[:, 1:NCH + 1, :, h, :])
                        for b in range(BC):
                            pss = sps.tile([1, T], f32, tag="pss", name="pss")
                            nc.tensor.matmul(
                                pss[:], qT[:, h, b:b + 1], KTh[:, :, :, b],
                                start=True, stop=True)
                            srow = swk.tile([1, T], f32, tag="srow",
                                            name="srow")
                            nc.scalar.copy(srow[:], pss[:])
                            nc.sync.dma_start(
                                out=sc[b * NH + h: b * NH + h + 1, :],
                                in_=srow[:])
                mx = vsp.tile([NH * BC, 1], f32, name="mx")
                nc.vector.tensor_reduce(mx[:], sc[:], axis=mybir.AxisListType.X,
                                        op=ALU.max)
                nmx = vsp.tile([NH * BC, 1], f32, name="nmx")
                nc.vector.tensor_scalar(nmx[:], mx[:], -1.0, None, op0=ALU.mult)
                ex = vsp.tile([NH * BC, T], f32, name="ex")
                sm = vsp.tile([NH * BC, 1], f32, name="sm")
                nc.scalar.activation(ex[:], sc[:], ACTF.Exp, bias=nmx[:],
                                     accum_out=sm[:])
                rsm = vsp.tile([NH * BC, 1], f32, name="rsm")
                nc.vector.reciprocal_approx_fast(out=rsm[:], in_=sm[:])
                en = vsp.tile([NH * BC, T], f32, name="en")
                nc.vector.tensor_scalar(en[:], ex[:], rsm[:], None,
                                        op0=ALU.mult)
                eT = []
                with tc.tile_pool(name="etps", bufs=2, space="PSUM") as eps_p:
                    for kc in range(NT):
                        pse = eps_p.tile([P, NH * BC], f32, tag="pse",
                                         name="pse")
                        nc.tensor.transpose(pse[:], en[:, kc * P:(kc + 1) * P],
                                            ident[:])
                        esb = vsp.tile([P, NH * BC], f32, name=f"eT{kc}",
                                       tag=f"eT{kc}")
                        nc.scalar.copy(esb[:], pse[:])
                        eT.append(esb)

                # ---- V (recomputed per example) + o ----
                psO_sb = vsp.tile([P, HC, BC], f32, name="psO_sb")
                with tc.tile_pool(name="vps", bufs=2, space="PSUM") as vps, \
                     tc.tile_pool(name="vwk", bufs=2) as vwk, \
                     tc.tile_pool(name="hbp", bufs=1) as hbp, \
                     tc.tile_pool(name="ops", bufs=1, space="PSUM") as ops_:
                    psO = ops_.tile([P, HC, BC], f32, tag="psO", name="psO")
                    BH = BC // 2
                    for bh in range(2):
                        # bulk-contiguous load of h1 for 8 examples
                        hball = hbp.tile([P, HC, T, BH], f32, tag="hball",
                                         name="hball")
                        nc.sync.dma_start(
                            out=hball[:],
                            in_=hD[:, :, CH:TP, bh * BH:(bh + 1) * BH])
                        for b8 in range(BH):
                            b = bh * BH + b8
                            Vb = vwk.tile([P, NT * H], f32, tag="Vb",
                                          name="Vb")
                            for tcc in range(NT):
                                psV = vps.tile([P, H], f32, tag="psV",
                                               name="psV")
                                for kc in range(HC):
                                    nc.tensor.matmul(
                                        psV[:],
                                        hball[:, kc, tcc * P:(tcc + 1) * P,
                                              b8],
                                        wa["wvT"][:, kc * H:(kc + 1) * H],
                                        start=(kc == 0), stop=(kc == HC - 1))
                                t2 = vwk.tile([P, H], f32, tag="t2v",
                                              name="t2")
                                nc.vector.scalar_tensor_tensor(
                                    t2[:], wa["rsv_flat"][:],
                                    m1T[:, tcc, b:b + 1],
                                    psV[:], op0=ALU.mult, op1=ALU.add)
                                f_ = vwk.tile([P, H], f32, tag="f_v",
                                              name="f_")
                                nc.vector.tensor_scalar(
                                    f_[:], t2[:], r1T[:, tcc, b:b + 1], None,
                                    op0=ALU.mult)
                                nc.vector.tensor_add(
                                    Vb[:, tcc * H:(tcc + 1) * H], f_[:],
                                    wa["bv_flat"][:])
                            for h in range(NH):
                                for kc in range(NT):
                                    nc.tensor.matmul(
                                        psO[:, h, b:b + 1],
                                        Vb[:, kc * H + h * HD:
                                           kc * H + (h + 1) * HD],
                                        eT[kc][:, b * NH + h: b * NH + h + 1],
                                        start=(kc == 0), stop=(kc == NT - 1))
                    nc.scalar.copy(psO_sb[:], psO[:])

                # ---- head ----
                with tc.tile_pool(name="hps", bufs=1, space="PSUM") as hps, \
                     tc.tile_pool(name="hc_", bufs=1) as hcp:
                    psAO = hps.tile([P, HC, BC], f32, tag="psAO", name="psAO")
                    mmT(psAO, wa["woT"], psO_sb, HC)
                    ao = hcp.tile([P, HC, BC], f32, name="ao")
                    for hc in range(HC):
                        nc.scalar.activation(ao[:, hc], psAO[:, hc],
                                             ACTF.Identity,
                                             bias=wa["b_o"][:, hc:hc + 1])
                    psP1 = hps.tile([P, 2, BC], f32, tag="psP1", name="psP1")
                    for hc in range(2):
                        for kc in range(HC):
                            nc.tensor.matmul(
                                psP1[:, hc],
                                wa["p1T"][:, kc * (H // 2) + hc * P:
                                          kc * (H // 2) + hc * P + P],
                                ao[:, kc],
                                start=(kc == 0), stop=(kc == HC - 1))
                    h1_ = hcp.tile([P, 2, BC], f32, name="h1_")
                    for hc in range(2):
                        nc.scalar.activation(h1_[:, hc], psP1[:, hc], ACTF.Relu,
                                             bias=wa["b_p1"][:, hc:hc + 1])
                    psP2 = hps.tile([P, 2, BC], f32, tag="psP2", name="psP2")
                    for hc in range(2):
                        for kc in range(2):
                            nc.tensor.matmul(
                                psP2[:, hc],
                                wa["p2T"][:, kc * OUT + hc * P:
                                          kc * OUT + hc * P + P],
                                h1_[:, kc],
                                start=(kc == 0), stop=(kc == 1))
                    outT = hcp.tile([P, 2, BC], f32, name="outT")
                    for hc in range(2):
                        nc.scalar.activation(outT[:, hc], psP2[:, hc],
                                             ACTF.Identity,
                                             bias=wa["b_p2"][:, hc:hc + 1])
                    for c in range(2):
                        nc.sync.dma_start(
                            out=out_p[:, c * P:(c + 1) * P].rearrange(
                                "b p -> p b"),
                            in_=outT[:, c])

    nc.finalize()
    return nc


# ---------------------------------------------------------------- host driver

def _prep_inputs(inputs):
    d = {k: np.asarray(v, np.float32) for k, v in inputs.items()}
    sqh = np.float32(1.0 / np.sqrt(HD))

    wi1, bi1, rs_i1 = _fold3(d["Win1_w"], d["Win1_b"], d["ln0_w"], d["ln0_b"])
    wx1, bx1, rs_x1 = _fold3(d["tau1a_w"][:, :H], d["tau1a_b"],
                             d["ln0_w"], d["ln0_b"])
    ab = d["attn_in_b"]
    wq, bq, _ = _fold3(d["attn_in_w"][0:H] * sqh, ab[0:H] * sqh,
                       d["ln1_w"], d["ln1_b"])
    wk, bk, rs_k = _fold3(d["attn_in_w"][H:2 * H], ab[H:2 * H],
                          d["ln1_w"], d["ln1_b"])
    wv, bv, rs_v = _fold3(d["attn_in_w"][2 * H:3 * H], ab[2 * H:3 * H],
                          d["ln1_w"], d["ln1_b"])

    import ml_dtypes

    def zl(shape):
        return np.zeros(shape, np.float32)

    SH = {name: shape for name, shape in [
        ("x_T", (P, IN // P, TP * BC)),
        ("winT_in", (P, (IN // P) * H)), ("tauaxT_in", (P, (IN // P) * H)),
        ("b_iin", (P, HC)), ("b_xin", (P, HC)),
        ("pAT", (P, HC * H)), ("pBT", (P, HC * H)),
        ("nrs_pA", (P, HC)), ("nrs_pB", (P, HC)),
        ("b_pA", (P, HC)), ("b_pB", (P, HC)),
        ("wqT", (P, HC * H)), ("woT", (P, HC * H)), ("wvT", (P, HC * H)),
        ("p1T", (P, HC * (H // 2))), ("p2T", (P, 2 * OUT)),
        ("b_q", (P, HC)), ("b_o", (P, HC)), ("b_p1", (P, 2)), ("b_p2", (P, 2)),
        ("rsv_flat", (P, H)), ("bv_flat", (P, H)),
    ]}

    # ---- A role (even cores): layer-0 scan + (I1, X1) projections ----
    mapA = {
        "winT_in": _wT(d["Win0_w"]), "tauaxT_in": _wT(d["tau0a_w"][:, :IN]),
        "b_iin": _perH(d["Win0_b"]), "b_xin": _perH(d["tau0a_b"]),
        "wrecT": _wT(d["Wrec0_w"]), "tauavT": _wT(d["tau0a_w"][:, IN:]),
        "taubT": _wT(d["tau0b_w"]),
        "ngsbc": _bcast(-DT * d["gsyn0"]), "glbc": _bcast(DT * d["gleak0"]),
        "tbbbc": _bcast(d["tau0b_b"]),
        "pAT": _wT(wi1), "nrs_pA": _perH(-rs_i1), "b_pA": _perH(bi1),
        "pBT": _wT(wx1), "nrs_pB": _perH(-rs_x1), "b_pB": _perH(bx1),
        "wqT": zl(SH["wqT"]), "woT": zl(SH["woT"]), "wvT": zl(SH["wvT"]),
        "p1T": zl(SH["p1T"]), "p2T": zl(SH["p2T"]),
        "b_q": zl(SH["b_q"]), "b_o": zl(SH["b_o"]),
        "b_p1": zl(SH["b_p1"]), "b_p2": zl(SH["b_p2"]),
        "rsv_flat": zl(SH["rsv_flat"]), "bv_flat": zl(SH["bv_flat"]),
        "rolemask": np.zeros((P, 1), np.float32),
        "keep": np.ones((P, 1), np.float32),
    }
    # ---- B role (odd cores): layer-1 scan + K proj + attention ----
    mapB = {
        "winT_in": zl(SH["winT_in"]), "tauaxT_in": zl(SH["tauaxT_in"]),
        "b_iin": zl(SH["b_iin"]), "b_xin": zl(SH["b_xin"]),
        "wrecT": _wT(d["Wrec1_w"]), "tauavT": _wT(d["tau1a_w"][:, H:]),
        "taubT": _wT(d["tau1b_w"]),
        "ngsbc": _bcast(-DT * d["gsyn1"]), "glbc": _bcast(DT * d["gleak1"]),
        "tbbbc": _bcast(d["tau1b_b"]),
        "pAT": _wT(wk), "nrs_pA": _perH(-rs_k), "b_pA": _perH(bk),
        "pBT": zl(SH["pBT"]), "nrs_pB": zl(SH["nrs_pB"]),
        "b_pB": zl(SH["b_pB"]),
        "wqT": _wT(wq), "b_q": _perH(bq),
        "woT": _wT(d["attn_out_w"]), "b_o": _perH(d["attn_out_b"]),
        "wvT": _wT(wv),
        "rsv_flat": np.ascontiguousarray(
            np.broadcast_to((-rs_v)[None, :], (P, H))).astype(np.float32),
        "bv_flat": np.ascontiguousarray(
            np.broadcast_to(bv[None, :], (P, H))).astype(np.float32),
        "p1T": _wT(d["p1_w"]), "b_p1": _perH(d["p1_b"]),
        "p2T": _wT(d["p2_w"]), "b_p2": _perH(d["p2_b"]),
        "rolemask": np.ones((P, 1), np.float32),
        "keep": np.zeros((P, 1), np.float32),
    }
    for m in (mapA, mapB):
        for nm in ("wrecT", "tauavT", "taubT"):
            m[nm] = m[nm].astype(ml_dtypes.bfloat16)

    x = d["inputs"]
    xz = np.zeros((BC, TP, IN), np.float32)
    in_maps = []
    for pr in range(NPAIR):
        xp = np.zeros((BC, TP, IN), np.float32)
        xp[:, :T] = x[pr * BC:(pr + 1) * BC]
        mA = dict(mapA)
        mA["x_T"] = _xT(xp)
        mB = dict(mapB)
        mB["x_T"] = _xT(xz)
        in_maps.append(mA)
        in_maps.append(mB)
    return in_maps


def _run(inputs, trace=False):
    from concourse.bass_utils import run_bass_kernel_spmd
    if "nc" not in _CACHE:
        _CACHE["nc"] = _build()
    nc = _CACHE["nc"]
    in_maps = _prep_inputs(inputs)
    res = run_bass_kernel_spmd(nc, in_maps, list(range(NCORES)), trace=trace)
    full = np.zeros((B, OUT), np.float32)
    for pr in range(NPAIR):
        full[pr * BC:(pr + 1) * BC] = res.results[2 * pr + 1]["out"]
    return full, res


def kernel(**inputs):
    out, _ = _run(inputs, trace=False)
    return out

